# revision 2
# baseline (speedup 1.0000x reference)
"""Trainium2 Bass kernel for 3-layer GAT + graph pooling (nn_GATModel).

Edge-stream design (v2): instead of on-device dma_gather (Q7 desc-gen wall at
~8.4ns/row), the HOST replicates table rows into a per-edge slot-aligned
stream between launches (pure index glue / byte permutation, no FP math on
activations). The device then:
  - loads the h-stream as fp8 via SWDGE cast-DMA (fp8 in HBM -> fp16 in SBUF)
  - computes per-edge softmax weights alpha = exp(lrelu(as+ad)-M)/den
  - expands alpha to full head width (rotating DVE-int32 / ACT / GpSimd)
  - msg = h * alpha_full as a flat fp16 DVE multiply (2x mode)
  - slot-sum via PE identity matmuls + GpSimd halving tree
  - epilogue: h_next = out @ [W|Msrc|Mdst] + bias-row (as/ad fused into the
    matmul via host-precomputed param-only matrices)
Four launches: L0 (x@W1ext), agg1, agg2, agg3+pool. Host between launches
does index-glue only (table assembly + per-edge replication + fp8 encode).
"""

import os
import numpy as np
import ml_dtypes

import concourse.bacc as bacc
import concourse.tile as tile
import concourse.mybir as mybir
from concourse import bass, bass_utils
from contextlib import ExitStack

F16 = mybir.dt.float16
F32 = mybir.dt.float32
F8 = mybir.dt.float8e4
I32 = mybir.dt.int32

N_NODES = 50000
N_EDGES = 800000
N_GRAPHS = 512
HEADS = 4
HDIM = 64
NCORES = 8
NEG_SLOPE = 0.2
LOGIT_M = [6.0, 10.0, 10.0]
ZROW = N_NODES          # table row: h=0, as=+M (safe self for empty slots)
PROW = N_NODES + 1      # table row: h=0, as=-30000 (e == 0 padding)
TROWS = N_NODES + 2

_EXEC_NS = []


def _trace_on():
    return bool(os.environ.get("GAT_TRACE"))


def _install_profhook():
    import sys, types
    if "antenv.axon_hooks" in sys.modules:
        return True
    try:
        mod = types.ModuleType("antenv.axon_hooks")
        state = {}
        mod.set_axon_ntff_profile_hook = lambda h: state.update(h=h)
        mod.get_axon_ntff_profile_hook = lambda: state.get("h")
        sys.modules["antenv.axon_hooks"] = mod
        sys.path.insert(0, "/root/.axon_site/trn_agent_boot")
        import trn_boot
        mod.set_axon_ntff_profile_hook(
            trn_boot._ntff_profile_via_ctypes("/opt/axon/libaxon_pjrt.so")
        )
        return True
    except Exception:
        sys.modules.pop("antenv.axon_hooks", None)
        return False


# ---------------------------------------------------------------- host prep

def build_meta(edge_index):
    src = np.asarray(edge_index[0], dtype=np.int64)
    dst = np.asarray(edge_index[1], dtype=np.int64)
    deg = np.bincount(dst, minlength=N_NODES)

    cum = np.cumsum(deg + 1)
    total = cum[-1]
    bounds = [0]
    for c in range(1, NCORES):
        bounds.append(int(np.searchsorted(cum, total * c / NCORES)))
    bounds.append(N_NODES)

    NW = max((bounds[c + 1] - bounds[c] + 127) // 128 for c in range(NCORES))
    maxn = NW * 128

    # edges sorted by dst; slot of edge within its dst segment
    order = np.argsort(dst, kind="stable")
    src_s, dst_s = src[order], dst[order]
    starts = np.searchsorted(dst_s, np.arange(N_NODES))
    slot_e = np.arange(len(src)) - starts[dst_s] + 1  # slot 0 = self

    cores = []
    kT_cores = []
    for c in range(NCORES):
        n0, n1 = bounds[c], bounds[c + 1]
        nodes = np.arange(n0, n1)
        o = np.argsort(deg[nodes], kind="stable")[::-1]
        perm = np.full(maxn, -1, np.int64)
        perm[: n1 - n0] = nodes[o]
        kc = np.zeros(NW, np.int32)
        dpad = np.zeros(maxn, np.int64)
        real = perm >= 0
        dpad[real] = deg[perm[real]] + 1
        kc = dpad.reshape(NW, 128).max(1).astype(np.int32)
        cores.append(dict(n0=n0, n1=n1, perm=perm))
        kT_cores.append(np.maximum(kc, 1))
    kT = np.maximum.reduce(kT_cores)
    offs = np.concatenate([[0], np.cumsum(kT)]).astype(np.int64)
    TOT = int(offs[-1])

    for cd in cores:
        perm = cd["perm"]
        posof = np.full(N_NODES, -1, np.int64)
        real = perm >= 0
        posof[perm[real]] = np.nonzero(real)[0]
        idx = np.full((128, TOT), PROW, np.int32)
        # self slots
        pos = posof[perm[real]]
        w, p = pos // 128, pos % 128
        idx[p, offs[w]] = perm[real]
        # empty partitions: self -> ZROW
        epos = np.nonzero(~real)[0]
        idx[epos % 128, offs[epos // 128]] = ZROW
        # edges owned by this core
        sel = (dst_s >= cd["n0"]) & (dst_s < cd["n1"])
        pos = posof[dst_s[sel]]
        w, p = pos // 128, pos % 128
        idx[p, offs[w] + slot_e[sel]] = src_s[sel].astype(np.int32)
        cd["idx"] = idx

    return dict(NW=NW, kT=kT, offs=offs, TOT=TOT, cores=cores)


def build_pool(meta, batch):
    batch = np.asarray(batch, dtype=np.int64)
    NW = meta["NW"]
    for cd in meta["cores"]:
        perm = cd["perm"]
        gbase = int(batch[cd["n0"]])
        gspan = int(batch[cd["n1"] - 1]) - gbase + 1
        assert gspan <= 128
        oh = np.zeros((NW * 128, 128), np.float16)
        real = perm >= 0
        oh[np.nonzero(real)[0], batch[perm[real]] - gbase] = 1.0
        cd["pool_onehot"] = oh.reshape(NW, 128, 128)
        cd["gbase"] = gbase
    meta["gcounts"] = np.bincount(batch, minlength=N_GRAPHS).astype(np.float64)


def make_wext(W, a_s, a_d):
    """[Fin, 256] params -> [Fin, 264] f32 with as/ad columns fused."""
    W = np.asarray(W, np.float64)
    a_s = np.asarray(a_s, np.float64)
    a_d = np.asarray(a_d, np.float64)
    W3 = W.reshape(W.shape[0], HEADS, HDIM)
    ms = np.einsum("chd,hd->ch", W3, a_s)
    md = np.einsum("chd,hd->ch", W3, a_d)
    return np.concatenate([W, ms, md], axis=1).astype(np.float32)


def make_bext(b, a_s, a_d):
    b = np.asarray(b, np.float64)
    bs = np.einsum("hd,hd->h", b.reshape(HEADS, HDIM), np.asarray(a_s, np.float64))
    bd = np.einsum("hd,hd->h", b.reshape(HEADS, HDIM), np.asarray(a_d, np.float64))
    return np.concatenate([b, bs, bd]).astype(np.float32)


# ---------------------------------------------------------------- device util

def _ap(t_ap, off_elems, dims):
    return bass.AP(t_ap.tensor, t_ap.offset + off_elems, dims)


# ---------------------------------------------------------------- programs

def build_l0(meta):
    NW = meta["NW"]
    nc = bacc.Bacc("TRN2", target_bir_lowering=False, debug=False, num_devices=NCORES)
    xT = nc.dram_tensor("xT", [128, NW * 128], F16, kind="ExternalInput").ap()
    Wext = nc.dram_tensor("Wext", [128, 264], F16, kind="ExternalInput").ap()
    bext = nc.dram_tensor("bext", [1, 264], F16, kind="ExternalInput").ap()
    ones1 = nc.dram_tensor("ones1", [1, 128], F16, kind="ExternalInput").ap()
    hout = nc.dram_tensor("hout", [NW, 128, 264], F16, kind="ExternalOutput").ap()

    with ExitStack() as ctx:
        tc = ctx.enter_context(tile.TileContext(nc))
        cpool = ctx.enter_context(tc.tile_pool(name="c", bufs=1))
        spool = ctx.enter_context(tc.tile_pool(name="s", bufs=4))
        pspool = ctx.enter_context(tc.tile_pool(name="ps", bufs=3, space="PSUM"))
        W_s = cpool.tile([128, 264], F16)
        nc.sync.dma_start(W_s[:], Wext[:])
        b_s = cpool.tile([1, 264], F16)
        nc.sync.dma_start(b_s[:], bext[:])
        o_s = cpool.tile([1, 128], F16)
        nc.sync.dma_start(o_s[:], ones1[:])
        xT_s = cpool.tile([128, NW * 128], F16)
        nc.sync.dma_start(xT_s[:], xT[:])
        for w in range(NW):
            hp = pspool.tile([128, 264], F32, tag="hp")
            nc.tensor.matmul(hp[:], lhsT=xT_s[:, w * 128:(w + 1) * 128], rhs=W_s[:],
                             start=True, stop=False)
            nc.tensor.matmul(hp[:], lhsT=o_s[:], rhs=b_s[:], start=False, stop=True)
            ho = spool.tile([128, 264], F16, tag="ho")
            nc.scalar.copy(ho[:], hp[:])
            nc.sync.dma_start(hout[w], ho[:])
    nc.compile()
    return nc


def build_agg(meta, last):
    NW, kT, offs, TOT = meta["NW"], meta["kT"], meta["offs"], meta["TOT"]
    nc = bacc.Bacc("TRN2", target_bir_lowering=False, debug=False, num_devices=NCORES)
    stream = nc.dram_tensor("stream", [128, TOT * 256], F8, kind="ExternalInput").ap()
    asst = nc.dram_tensor("asst", [128, TOT * 4], F16, kind="ExternalInput").ap()
    adin = nc.dram_tensor("adin", [128, NW * 4], F16, kind="ExternalInput").ap()
    mshift = nc.dram_tensor("mshift", [128, 1], F32, kind="ExternalInput").ap()
    ident16 = nc.dram_tensor("ident16", [128, 128], F16, kind="ExternalInput").ap()
    if not last:
        Wext = nc.dram_tensor("Wext", [128, 2 * 264], F16, kind="ExternalInput").ap()
        bext = nc.dram_tensor("bext", [1, 264], F16, kind="ExternalInput").ap()
        ones1 = nc.dram_tensor("ones1", [1, 128], F16, kind="ExternalInput").ap()
        hout = nc.dram_tensor("hout", [NW, 128, 264], F16, kind="ExternalOutput").ap()
    else:
        onehot = nc.dram_tensor("onehot", [NW, 128, 128], F16, kind="ExternalInput").ap()
        fcw = nc.dram_tensor("fcw", [128, 256], F32, kind="ExternalInput").ap()
        pout = nc.dram_tensor("pout", [128, 1], F32, kind="ExternalOutput").ap()

    kmax = int(kT.max())

    with ExitStack() as ctx:
        tc = ctx.enter_context(tile.TileContext(nc))
        cpool = ctx.enter_context(tc.tile_pool(name="c", bufs=1))
        gpool = ctx.enter_context(tc.tile_pool(name="g", bufs=3))
        apool = ctx.enter_context(tc.tile_pool(name="a", bufs=2))
        spool = ctx.enter_context(tc.tile_pool(name="s", bufs=4))
        pspool = ctx.enter_context(tc.tile_pool(name="ps", bufs=2, space="PSUM"))
        pxpool = ctx.enter_context(tc.tile_pool(name="px", bufs=3, space="PSUM"))
        pppool = ctx.enter_context(tc.tile_pool(name="pp", bufs=1, space="PSUM"))

        as_s = cpool.tile([128, TOT * 4], F16)
        nc.sync.dma_start(as_s[:], asst[:])
        ad_s = cpool.tile([128, NW * 4], F16)
        nc.sync.dma_start(ad_s[:], adin[:])
        msh_s = cpool.tile([128, 1], F32)
        nc.sync.dma_start(msh_s[:], mshift[:])
        id16 = cpool.tile([128, 128], F16)
        nc.sync.dma_start(id16[:], ident16[:])
        alslope = cpool.tile([128, 1], F32)
        nc.vector.memset(alslope[:], NEG_SLOPE)
        if not last:
            W_s = cpool.tile([128, 2 * 264], F16)
            nc.sync.dma_start(W_s[:], Wext[:])
            b_s = cpool.tile([1, 264], F16)
            nc.sync.dma_start(b_s[:], bext[:])
            o_s = cpool.tile([1, 128], F16)
            nc.sync.dma_start(o_s[:], ones1[:])
        else:
            fcw_s = cpool.tile([128, 256], F32)
            nc.sync.dma_start(fcw_s[:], fcw[:])
            pp = pppool.tile([128, 256], F32)

        for w in range(NW):
            k = int(kT[w])
            o4 = int(offs[w]) * 4
            # h-stream: fp8 in HBM -> fp16 in SBUF via SWDGE cast DMA
            g = gpool.tile([128, k * 256], F16, tag="g")
            nc.gpsimd.dma_start(g[:], stream[:, int(offs[w]) * 256:(int(offs[w]) + k) * 256])

            # logits lg = as + ad (f32), lr = leaky_relu
            lg = spool.tile([128, kmax * 4], F32, tag="lg")
            as_ap = as_s[:, o4:o4 + k * 4]
            ad_b = _ap(ad_s[:], w * 4, [list(ad_s[:].ap[0]), [0, k], [1, 4]])
            as_v = _ap(as_s[:], o4, [list(as_s[:].ap[0]), [4, k], [1, 4]])
            nc.vector.tensor_tensor(
                out=_ap(lg[:], 0, [list(lg[:].ap[0]), [4, k], [1, 4]]),
                in0=as_v, in1=ad_b, op=mybir.AluOpType.add)
            lr = spool.tile([128, kmax * 4], F32, tag="lr")
            nc.scalar.activation(lr[:, :k * 4], lg[:, :k * 4],
                                 mybir.ActivationFunctionType.Prelu,
                                 alpha=alslope[:])
            # e = exp(lr - M)  (ACT)
            e = spool.tile([128, kmax * 4], F16, tag="e")
            nc.scalar.activation(e[:, :k * 4], lr[:, :k * 4],
                                 mybir.ActivationFunctionType.Exp,
                                 bias=msh_s[:], scale=1.0)
            # den + recip
            den = spool.tile([128, 4], F32, tag="den")
            nc.vector.reduce_sum(
                den[:], _ap(e[:], 0, [list(e[:].ap[0]), [1, 4], [4, k]]),
                axis=mybir.AxisListType.X)
            rdeb = spool.tile([128, 4], F32, tag="rdeb")
            nc.vector.reciprocal(rdeb[:], den[:])
            # etwin: alpha duplicated pairwise = e * recip (one 3-dim TT)
            etw = spool.tile([128, kmax * 8], F16, tag="etw")
            rdeb_b3 = _ap(rdeb[:], 0, [list(rdeb[:].ap[0]), [0, k], [1, 4], [0, 2]])
            e_v3 = _ap(e[:], 0, [list(e[:].ap[0]), [4, k], [1, 4], [0, 2]])
            nc.vector.tensor_tensor(
                out=_ap(etw[:], 0, [list(etw[:].ap[0]), [8, k], [2, 4], [1, 2]]),
                in0=e_v3, in1=rdeb_b3, op=mybir.AluOpType.mult)
            # alpha expansion to [k*256] f16: DVE-int32 slots [0,kd), ACT rest
            af = apool.tile([128, kmax * 256], F16, tag="af")
            etw_i = etw[:].bitcast(I32)
            af_i = af[:].bitcast(I32)
            kd = max(1, int(k * 0.45))
            nc.vector.tensor_copy(
                _ap(af_i, 0, [list(af_i.ap[0]), [32, kd * 4], [1, 32]]),
                _ap(etw_i, 0, [list(etw_i.ap[0]), [1, kd * 4], [0, 32]]))
            if k > kd:
                nc.scalar.copy(
                    _ap(af[:], kd * 256,
                        [list(af[:].ap[0]), [256, k - kd], [64, 4], [1, 64]]),
                    _ap(etw[:], kd * 8,
                        [list(etw[:].ap[0]), [8, k - kd], [2, 4], [0, 64]]))
            # msg = h * alpha (flat fp16, 2x mode), in place
            nc.vector.tensor_tensor(out=g[:, :k * 256], in0=g[:, :k * 256],
                                    in1=af[:, :k * 256], op=mybir.AluOpType.mult)
            # slot-sum: GpSimd tree chunk + PE matmuls
            kG = k // 3 if k >= 9 else 0
            g3 = g[:].rearrange("p (k d) -> p k d", d=256)
            if kG:
                a0, n = k - kG, kG
                while n > 1:
                    h = n // 2
                    nc.gpsimd.tensor_tensor(
                        out=g[:, a0 * 256:(a0 + h) * 256],
                        in0=g[:, a0 * 256:(a0 + h) * 256],
                        in1=g[:, (a0 + n - h) * 256:(a0 + n) * 256],
                        op=mybir.AluOpType.add)
                    n -= h
            kPE = k - kG + (1 if kG else 0)
            ps = pspool.tile([128, 256], F32, tag="ps")
            for t in range(kPE):
                nc.tensor.matmul(ps[:], lhsT=id16[:], rhs=g3[:, t if t < k - kG else k - kG, :],
                                 start=(t == 0), stop=(t == kPE - 1))
            # out1 = ps (already alpha-normalized); cast f16
            of = spool.tile([128, 256], F16, tag="of")
            nc.scalar.copy(of[:], ps[:])
            if not last:
                outT = spool.tile([128, 256], F16, tag="outT")
                for q in range(2):
                    pt = pxpool.tile([128, 128], F16, tag="pt")
                    nc.tensor.transpose(pt[:], of[:, q * 128:(q + 1) * 128], id16[:])
                    if q == 0:
                        nc.scalar.copy(outT[:, q * 128:(q + 1) * 128], pt[:])
                    else:
                        nc.vector.tensor_copy(outT[:, q * 128:(q + 1) * 128], pt[:])
                hp = pxpool.tile([128, 264], F32, tag="hp")
                for q in range(2):
                    nc.tensor.matmul(hp[:], lhsT=outT[:, q * 128:(q + 1) * 128],
                                     rhs=W_s[:, q * 264:(q + 1) * 264],
                                     start=(q == 0), stop=False)
                nc.tensor.matmul(hp[:], lhsT=o_s[:], rhs=b_s[:], start=False, stop=True)
                ho = spool.tile([128, 264], F16, tag="ho")
                nc.scalar.copy(ho[:], hp[:])
                nc.sync.dma_start(hout[w], ho[:])
            else:
                ohw = spool.tile([128, 128], F16, tag="ohw")
                nc.sync.dma_start(ohw[:], onehot[w])
                nc.tensor.matmul(pp[:], lhsT=ohw[:], rhs=of[:],
                                 start=(w == 0), stop=(w == NW - 1),
                                 skip_group_check=True)
        if last:
            fm = spool.tile([128, 256], F32, tag="fm")
            nc.vector.tensor_tensor(out=fm[:], in0=pp[:], in1=fcw_s[:],
                                    op=mybir.AluOpType.mult)
            pv = spool.tile([128, 1], F32, tag="pv")
            nc.vector.reduce_sum(pv[:], fm[:], axis=mybir.AxisListType.X)
            nc.sync.dma_start(pout[:], pv[:])
    nc.compile()
    return nc


# ---------------------------------------------------------------- run

def _run(nc, in_maps):
    trace = _trace_on() and _install_profhook()
    res = bass_utils.run_bass_kernel_spmd(
        nc, in_maps=in_maps, core_ids=list(range(NCORES)), trace=trace
    )
    if _trace_on():
        _EXEC_NS.append(res.exec_time_ns)
    return res


def kernel(x, edge_index, batch, W1, a_src1, a_dst1, b1, W2, a_src2, a_dst2, b2,
           W3, a_src3, a_dst3, b3, fc_W, fc_b):
    _EXEC_NS.clear()
    x = np.asarray(x, np.float32)
    edge_index = np.asarray(edge_index)
    batch = np.asarray(batch)
    meta = build_meta(edge_index)
    build_pool(meta, batch)
    NW, TOT = meta["NW"], meta["TOT"]
    id16 = np.eye(128, dtype=np.float16)
    ones1 = np.ones((1, 128), np.float16)

    wext1 = make_wext(W1, a_src1, a_dst1).astype(np.float16)
    bext1 = make_bext(b1, a_src1, a_dst1).astype(np.float16).reshape(1, 264)
    nc0 = build_l0(meta)
    in0 = []
    for cd in meta["cores"]:
        xp = np.zeros((NW * 128, 128), np.float16)
        real = cd["perm"] >= 0
        xp[real] = x[cd["perm"][real]].astype(np.float16)
        in0.append({"xT": np.ascontiguousarray(xp.T), "Wext": wext1, "bext": bext1,
                    "ones1": ones1})
    r0 = _run(nc0, in0)
    houts = [r0.results[c]["hout"].reshape(NW * 128, 264) for c in range(NCORES)]

    nc_mid = build_agg(meta, last=False)
    nc_last = build_agg(meta, last=True)

    wexts = [make_wext(W2, a_src2, a_dst2).astype(np.float16),
             make_wext(W3, a_src3, a_dst3).astype(np.float16), None]
    bexts = [make_bext(b2, a_src2, a_dst2).astype(np.float16).reshape(1, 264),
             make_bext(b3, a_src3, a_dst3).astype(np.float16).reshape(1, 264), None]

    for li in range(3):
        last = li == 2
        # assemble global tables from per-core houts
        tab_h = np.zeros((TROWS, 256), np.float32)
        tab_as = np.zeros((TROWS, 4), np.float16)
        for cd, h in zip(meta["cores"], houts):
            real = cd["perm"] >= 0
            tab_h[cd["perm"][real]] = h[real, 0:256].astype(np.float32)
            tab_as[cd["perm"][real]] = h[real, 256:260]
        tab_as[ZROW] = np.float16(LOGIT_M[li])
        tab_as[PROW] = np.float16(-30000.0)
        tab8 = np.clip(tab_h, -240, 240).astype(ml_dtypes.float8_e4m3fn)

        ims = []
        for c, cd in enumerate(meta["cores"]):
            stream = tab8[cd["idx"]].reshape(128, TOT * 256)
            asst = tab_as[cd["idx"]].reshape(128, TOT * 4)
            adin = np.ascontiguousarray(
                houts[c].reshape(NW, 128, 264)[:, :, 260:264]
                .transpose(1, 0, 2).reshape(128, NW * 4)).astype(np.float16)
            im = {"stream": stream, "asst": asst, "adin": adin,
                  "mshift": np.full((128, 1), -LOGIT_M[li], np.float32),
                  "ident16": id16}
            if not last:
                W2c = wexts[li]  # [256, 264]
                im["Wext"] = np.ascontiguousarray(
                    np.concatenate([W2c[0:128, :], W2c[128:256, :]], axis=1))
                im["bext"] = bexts[li]
                im["ones1"] = ones1
            else:
                im["onehot"] = cd["pool_onehot"].astype(np.float16)
                im["fcw"] = np.tile(np.asarray(fc_W, np.float32).reshape(1, 256), (128, 1))
            ims.append(im)
        rr = _run(nc_mid if not last else nc_last, ims)
        if not last:
            houts = [rr.results[c]["hout"].reshape(NW * 128, 264) for c in range(NCORES)]
        else:
            outv = np.zeros(N_GRAPHS, np.float64)
            for c, cd in enumerate(meta["cores"]):
                pv = rr.results[c]["pout"].reshape(128)
                gb = cd["gbase"]
                hi = min(128, N_GRAPHS - gb)
                outv[gb:gb + hi] += pv[:hi]
            bias_fc = float(np.asarray(b3, np.float64) @ np.asarray(fc_W, np.float64).reshape(-1))
            outv += meta["gcounts"] * bias_fc
            outv += float(np.asarray(fc_b, np.float64).reshape(()))
    return outv.reshape(N_GRAPHS, 1).astype(np.float32)


# revision 4
# speedup vs baseline: 1.1096x; 1.1096x over previous
"""Trainium2 Bass kernel for 3-layer GAT + graph pooling (nn_GATModel).

Edge-stream design (v2): instead of on-device dma_gather (Q7 desc-gen wall at
~8.4ns/row), the HOST replicates table rows into a per-edge slot-aligned
stream between launches (pure index glue / byte permutation, no FP math on
activations). The device then:
  - loads the h-stream as fp8 via SWDGE cast-DMA (fp8 in HBM -> fp16 in SBUF)
  - computes per-edge softmax weights alpha = exp(lrelu(as+ad)-M)/den
  - expands alpha to full head width (rotating DVE-int32 / ACT / GpSimd)
  - msg = h * alpha_full as a flat fp16 DVE multiply (2x mode)
  - slot-sum via PE identity matmuls + GpSimd halving tree
  - epilogue: h_next = out @ [W|Msrc|Mdst] + bias-row (as/ad fused into the
    matmul via host-precomputed param-only matrices)
Four launches: L0 (x@W1ext), agg1, agg2, agg3+pool. Host between launches
does index-glue only (table assembly + per-edge replication + fp8 encode).
"""

import os
import numpy as np
import ml_dtypes

import concourse.bacc as bacc
import concourse.tile as tile
import concourse.mybir as mybir
from concourse import bass, bass_utils
from contextlib import ExitStack

F16 = mybir.dt.float16
F32 = mybir.dt.float32
F8 = mybir.dt.float8e4
I32 = mybir.dt.int32

N_NODES = 50000
N_EDGES = 800000
N_GRAPHS = 512
HEADS = 4
HDIM = 64
NCORES = 8
NEG_SLOPE = 0.2
LOGIT_M = [6.0, 10.0, 10.0]
ZROW = N_NODES          # table row: h=0, as=+M (safe self for empty slots)
PROW = N_NODES + 1      # table row: h=0, as=-30000 (e == 0 padding)
TROWS = N_NODES + 2

_EXEC_NS = []


def _trace_on():
    return bool(os.environ.get("GAT_TRACE"))


def _install_profhook():
    import sys, types
    if "antenv.axon_hooks" in sys.modules:
        return True
    try:
        mod = types.ModuleType("antenv.axon_hooks")
        state = {}
        mod.set_axon_ntff_profile_hook = lambda h: state.update(h=h)
        mod.get_axon_ntff_profile_hook = lambda: state.get("h")
        sys.modules["antenv.axon_hooks"] = mod
        sys.path.insert(0, "/root/.axon_site/trn_agent_boot")
        import trn_boot
        mod.set_axon_ntff_profile_hook(
            trn_boot._ntff_profile_via_ctypes("/opt/axon/libaxon_pjrt.so")
        )
        return True
    except Exception:
        sys.modules.pop("antenv.axon_hooks", None)
        return False


# ---------------------------------------------------------------- host prep

def build_meta(edge_index):
    src = np.asarray(edge_index[0], dtype=np.int64)
    dst = np.asarray(edge_index[1], dtype=np.int64)
    deg = np.bincount(dst, minlength=N_NODES)

    cum = np.cumsum(deg + 1)
    total = cum[-1]
    bounds = [0]
    for c in range(1, NCORES):
        bounds.append(int(np.searchsorted(cum, total * c / NCORES)))
    bounds.append(N_NODES)

    NW = max((bounds[c + 1] - bounds[c] + 127) // 128 for c in range(NCORES))
    maxn = NW * 128

    # edges sorted by dst; slot of edge within its dst segment
    order = np.argsort(dst, kind="stable")
    src_s, dst_s = src[order], dst[order]
    starts = np.searchsorted(dst_s, np.arange(N_NODES))
    slot_e = np.arange(len(src)) - starts[dst_s] + 1  # slot 0 = self

    cores = []
    kT_cores = []
    for c in range(NCORES):
        n0, n1 = bounds[c], bounds[c + 1]
        nodes = np.arange(n0, n1)
        o = np.argsort(deg[nodes], kind="stable")[::-1]
        perm = np.full(maxn, -1, np.int64)
        perm[: n1 - n0] = nodes[o]
        kc = np.zeros(NW, np.int32)
        dpad = np.zeros(maxn, np.int64)
        real = perm >= 0
        dpad[real] = deg[perm[real]] + 1
        kc = dpad.reshape(NW, 128).max(1).astype(np.int32)
        cores.append(dict(n0=n0, n1=n1, perm=perm))
        kT_cores.append(np.maximum(kc, 1))
    kT = np.maximum.reduce(kT_cores)
    offs = np.concatenate([[0], np.cumsum(kT)]).astype(np.int64)
    TOT = int(offs[-1])

    for cd in cores:
        perm = cd["perm"]
        posof = np.full(N_NODES, -1, np.int64)
        real = perm >= 0
        posof[perm[real]] = np.nonzero(real)[0]
        idx = np.full((128, TOT), PROW, np.int32)
        # self slots
        pos = posof[perm[real]]
        w, p = pos // 128, pos % 128
        idx[p, offs[w]] = perm[real]
        # empty partitions: self -> ZROW
        epos = np.nonzero(~real)[0]
        idx[epos % 128, offs[epos // 128]] = ZROW
        # edges owned by this core
        sel = (dst_s >= cd["n0"]) & (dst_s < cd["n1"])
        pos = posof[dst_s[sel]]
        w, p = pos // 128, pos % 128
        idx[p, offs[w] + slot_e[sel]] = src_s[sel].astype(np.int32)
        cd["idx"] = idx

    return dict(NW=NW, kT=kT, offs=offs, TOT=TOT, cores=cores)


def build_pool(meta, batch):
    batch = np.asarray(batch, dtype=np.int64)
    NW = meta["NW"]
    for cd in meta["cores"]:
        perm = cd["perm"]
        gbase = int(batch[cd["n0"]])
        gspan = int(batch[cd["n1"] - 1]) - gbase + 1
        assert gspan <= 128
        oh = np.zeros((NW * 128, 128), np.float16)
        real = perm >= 0
        oh[np.nonzero(real)[0], batch[perm[real]] - gbase] = 1.0
        cd["pool_onehot"] = oh.reshape(NW, 128, 128)
        cd["gbase"] = gbase
    meta["gcounts"] = np.bincount(batch, minlength=N_GRAPHS).astype(np.float64)


def make_wext(W, a_s, a_d):
    """[Fin, 256] params -> [Fin, 264] f32 with as/ad columns fused."""
    W = np.asarray(W, np.float64)
    a_s = np.asarray(a_s, np.float64)
    a_d = np.asarray(a_d, np.float64)
    W3 = W.reshape(W.shape[0], HEADS, HDIM)
    ms = np.einsum("chd,hd->ch", W3, a_s)
    md = np.einsum("chd,hd->ch", W3, a_d)
    return np.concatenate([W, ms, md], axis=1).astype(np.float32)


def make_bext(b, a_s, a_d):
    b = np.asarray(b, np.float64)
    bs = np.einsum("hd,hd->h", b.reshape(HEADS, HDIM), np.asarray(a_s, np.float64))
    bd = np.einsum("hd,hd->h", b.reshape(HEADS, HDIM), np.asarray(a_d, np.float64))
    return np.concatenate([b, bs, bd]).astype(np.float32)


# ---------------------------------------------------------------- device util

def _ap(t_ap, off_elems, dims):
    return bass.AP(t_ap.tensor, t_ap.offset + off_elems, dims)


# ---------------------------------------------------------------- programs

def build_l0(meta):
    NW = meta["NW"]
    nc = bacc.Bacc("TRN2", target_bir_lowering=False, debug=False, num_devices=NCORES)
    xT = nc.dram_tensor("xT", [128, NW * 128], F16, kind="ExternalInput").ap()
    Wext = nc.dram_tensor("Wext", [128, 264], F16, kind="ExternalInput").ap()
    bext = nc.dram_tensor("bext", [1, 264], F16, kind="ExternalInput").ap()
    ones1 = nc.dram_tensor("ones1", [1, 128], F16, kind="ExternalInput").ap()
    hout = nc.dram_tensor("hout", [NW, 128, 264], F16, kind="ExternalOutput").ap()

    with ExitStack() as ctx:
        tc = ctx.enter_context(tile.TileContext(nc))
        cpool = ctx.enter_context(tc.tile_pool(name="c", bufs=1))
        spool = ctx.enter_context(tc.tile_pool(name="s", bufs=4))
        pspool = ctx.enter_context(tc.tile_pool(name="ps", bufs=3, space="PSUM"))
        W_s = cpool.tile([128, 264], F16)
        nc.sync.dma_start(W_s[:], Wext[:])
        b_s = cpool.tile([1, 264], F16)
        nc.sync.dma_start(b_s[:], bext[:])
        o_s = cpool.tile([1, 128], F16)
        nc.sync.dma_start(o_s[:], ones1[:])
        xT_s = cpool.tile([128, NW * 128], F16)
        nc.sync.dma_start(xT_s[:], xT[:])
        for w in range(NW):
            hp = pspool.tile([128, 264], F32, tag="hp")
            nc.tensor.matmul(hp[:], lhsT=xT_s[:, w * 128:(w + 1) * 128], rhs=W_s[:],
                             start=True, stop=False)
            nc.tensor.matmul(hp[:], lhsT=o_s[:], rhs=b_s[:], start=False, stop=True)
            ho = spool.tile([128, 264], F16, tag="ho")
            nc.scalar.copy(ho[:], hp[:])
            nc.sync.dma_start(hout[w], ho[:])
    nc.compile()
    return nc


def build_agg(meta, last):
    NW, kT, offs, TOT = meta["NW"], meta["kT"], meta["offs"], meta["TOT"]
    nc = bacc.Bacc("TRN2", target_bir_lowering=False, debug=False, num_devices=NCORES)
    stream = nc.dram_tensor("stream", [128, TOT * 256], F8, kind="ExternalInput").ap()
    asst = nc.dram_tensor("asst", [128, TOT * 4], F16, kind="ExternalInput").ap()
    adin = nc.dram_tensor("adin", [128, NW * 4], F16, kind="ExternalInput").ap()
    mshift = nc.dram_tensor("mshift", [128, 1], F32, kind="ExternalInput").ap()
    ident16 = nc.dram_tensor("ident16", [128, 128], F16, kind="ExternalInput").ap()
    if not last:
        Wext = nc.dram_tensor("Wext", [128, 2 * 264], F16, kind="ExternalInput").ap()
        bext = nc.dram_tensor("bext", [1, 264], F16, kind="ExternalInput").ap()
        ones1 = nc.dram_tensor("ones1", [1, 128], F16, kind="ExternalInput").ap()
        hout = nc.dram_tensor("hout", [NW, 128, 264], F16, kind="ExternalOutput").ap()
    else:
        onehot = nc.dram_tensor("onehot", [NW, 128, 128], F16, kind="ExternalInput").ap()
        fcw = nc.dram_tensor("fcw", [128, 256], F32, kind="ExternalInput").ap()
        pout = nc.dram_tensor("pout", [128, 1], F32, kind="ExternalOutput").ap()

    kmax = int(kT.max())

    with ExitStack() as ctx:
        tc = ctx.enter_context(tile.TileContext(nc))
        cpool = ctx.enter_context(tc.tile_pool(name="c", bufs=1))
        gpool = ctx.enter_context(tc.tile_pool(name="g", bufs=5))
        spool = ctx.enter_context(tc.tile_pool(name="s", bufs=6))
        pspool = ctx.enter_context(tc.tile_pool(name="ps", bufs=2, space="PSUM"))
        pxpool = ctx.enter_context(tc.tile_pool(name="px", bufs=3, space="PSUM"))
        pppool = ctx.enter_context(tc.tile_pool(name="pp", bufs=1, space="PSUM"))

        as_s = cpool.tile([128, TOT * 4], F16)
        nc.sync.dma_start(as_s[:], asst[:])
        ad_s = cpool.tile([128, NW * 4], F16)
        nc.sync.dma_start(ad_s[:], adin[:])
        msh_s = cpool.tile([128, 1], F32)
        nc.sync.dma_start(msh_s[:], mshift[:])
        id16 = cpool.tile([128, 128], F16)
        nc.sync.dma_start(id16[:], ident16[:])
        alslope = cpool.tile([128, 1], F32)
        nc.vector.memset(alslope[:], NEG_SLOPE)
        if not last:
            W_s = cpool.tile([128, 2 * 264], F16)
            nc.sync.dma_start(W_s[:], Wext[:])
            b_s = cpool.tile([1, 264], F16)
            nc.sync.dma_start(b_s[:], bext[:])
            o_s = cpool.tile([1, 128], F16)
            nc.sync.dma_start(o_s[:], ones1[:])
        else:
            fcw_s = cpool.tile([128, 256], F32)
            nc.sync.dma_start(fcw_s[:], fcw[:])
            pp = pppool.tile([128, 256], F32)

        worder = sorted(range(NW), key=lambda i: int(kT[i]))
        for wi, w in enumerate(worder):
            k = int(kT[w])
            o4 = int(offs[w]) * 4
            # h-stream: fp8 in HBM -> fp16 in SBUF via SWDGE cast DMA
            g = gpool.tile([128, k * 256], F16, tag="g")
            nc.gpsimd.dma_start(g[:], stream[:, int(offs[w]) * 256:(int(offs[w]) + k) * 256])

            # logits lg = as + ad (f32), lr = leaky_relu
            lg = spool.tile([128, kmax * 4], F32, tag="lg")
            as_ap = as_s[:, o4:o4 + k * 4]
            ad_b = _ap(ad_s[:], w * 4, [list(ad_s[:].ap[0]), [0, k], [1, 4]])
            as_v = _ap(as_s[:], o4, [list(as_s[:].ap[0]), [4, k], [1, 4]])
            nc.vector.tensor_tensor(
                out=_ap(lg[:], 0, [list(lg[:].ap[0]), [4, k], [1, 4]]),
                in0=as_v, in1=ad_b, op=mybir.AluOpType.add)
            lr = spool.tile([128, kmax * 4], F32, tag="lr")
            nc.scalar.activation(lr[:, :k * 4], lg[:, :k * 4],
                                 mybir.ActivationFunctionType.Prelu,
                                 alpha=alslope[:])
            # e = exp(lr - M)  (ACT)
            e = spool.tile([128, kmax * 4], F16, tag="e")
            nc.scalar.activation(e[:, :k * 4], lr[:, :k * 4],
                                 mybir.ActivationFunctionType.Exp,
                                 bias=msh_s[:], scale=1.0)
            # den + recip
            den = spool.tile([128, 4], F32, tag="den")
            nc.vector.reduce_sum(
                den[:], _ap(e[:], 0, [list(e[:].ap[0]), [1, 4], [4, k]]),
                axis=mybir.AxisListType.X)
            rdeb = spool.tile([128, 4], F32, tag="rdeb")
            nc.vector.reciprocal(rdeb[:], den[:])
            # etwin: alpha duplicated pairwise = e * recip (one 3-dim TT)
            etw = spool.tile([128, kmax * 8], F16, tag="etw")
            rdeb_b3 = _ap(rdeb[:], 0, [list(rdeb[:].ap[0]), [0, k], [1, 4], [0, 2]])
            e_v3 = _ap(e[:], 0, [list(e[:].ap[0]), [4, k], [1, 4], [0, 2]])
            nc.vector.tensor_tensor(
                out=_ap(etw[:], 0, [list(etw[:].ap[0]), [8, k], [2, 4], [1, 2]]),
                in0=e_v3, in1=rdeb_b3, op=mybir.AluOpType.mult)
            # msg = h * alpha, in place: pair-AP trick keeps DVE 2x mode
            # (innermost step-1 over the duplicated alpha pairs in etw)
            ge = _ap(g[:], 0, [list(g[:].ap[0]), [64, k * 4], [2, 32], [1, 2]])
            ee = _ap(etw[:], 0, [list(etw[:].ap[0]), [2, k * 4], [0, 32], [1, 2]])
            nc.vector.tensor_tensor(out=ge, in0=ge, in1=ee, op=mybir.AluOpType.mult)
            # slot-sum: GpSimd tree chunk + PE matmuls
            kG = max(k // 5, 2) if k >= 10 else 0
            g3 = g[:].rearrange("p (k d) -> p k d", d=256)
            if kG:
                a0, n = k - kG, kG
                while n > 1:
                    h = n // 2
                    nc.gpsimd.tensor_tensor(
                        out=g[:, a0 * 256:(a0 + h) * 256],
                        in0=g[:, a0 * 256:(a0 + h) * 256],
                        in1=g[:, (a0 + n - h) * 256:(a0 + n) * 256],
                        op=mybir.AluOpType.add)
                    n -= h
            kPE = k - kG + (1 if kG else 0)
            ps = pspool.tile([128, 256], F32, tag="ps")
            for t in range(kPE):
                nc.tensor.matmul(ps[:], lhsT=id16[:], rhs=g3[:, t if t < k - kG else k - kG, :],
                                 start=(t == 0), stop=(t == kPE - 1))
            # out1 = ps (already alpha-normalized); cast f16
            of = spool.tile([128, 256], F16, tag="of")
            nc.scalar.copy(of[:], ps[:])
            if not last:
                outT = spool.tile([128, 256], F16, tag="outT")
                for q in range(2):
                    pt = pxpool.tile([128, 128], F16, tag="pt")
                    nc.tensor.transpose(pt[:], of[:, q * 128:(q + 1) * 128], id16[:])
                    nc.scalar.copy(outT[:, q * 128:(q + 1) * 128], pt[:])
                hp = pxpool.tile([128, 264], F32, tag="hp")
                for q in range(2):
                    nc.tensor.matmul(hp[:], lhsT=outT[:, q * 128:(q + 1) * 128],
                                     rhs=W_s[:, q * 264:(q + 1) * 264],
                                     start=(q == 0), stop=False)
                nc.tensor.matmul(hp[:], lhsT=o_s[:], rhs=b_s[:], start=False, stop=True)
                ho = spool.tile([128, 264], F16, tag="ho")
                nc.scalar.copy(ho[:], hp[:])
                nc.sync.dma_start(hout[w], ho[:])
            else:
                ohw = spool.tile([128, 128], F16, tag="ohw")
                nc.sync.dma_start(ohw[:], onehot[w])
                nc.tensor.matmul(pp[:], lhsT=ohw[:], rhs=of[:],
                                 start=(wi == 0), stop=(wi == NW - 1),
                                 skip_group_check=True)
        if last:
            fm = spool.tile([128, 256], F32, tag="fm")
            nc.vector.tensor_tensor(out=fm[:], in0=pp[:], in1=fcw_s[:],
                                    op=mybir.AluOpType.mult)
            pv = spool.tile([128, 1], F32, tag="pv")
            nc.vector.reduce_sum(pv[:], fm[:], axis=mybir.AxisListType.X)
            nc.sync.dma_start(pout[:], pv[:])
    nc.compile()
    return nc


# ---------------------------------------------------------------- run

def _run(nc, in_maps):
    trace = _trace_on() and _install_profhook()
    res = bass_utils.run_bass_kernel_spmd(
        nc, in_maps=in_maps, core_ids=list(range(NCORES)), trace=trace
    )
    if _trace_on():
        _EXEC_NS.append(res.exec_time_ns)
    return res


def kernel(x, edge_index, batch, W1, a_src1, a_dst1, b1, W2, a_src2, a_dst2, b2,
           W3, a_src3, a_dst3, b3, fc_W, fc_b):
    _EXEC_NS.clear()
    x = np.asarray(x, np.float32)
    edge_index = np.asarray(edge_index)
    batch = np.asarray(batch)
    meta = build_meta(edge_index)
    build_pool(meta, batch)
    NW, TOT = meta["NW"], meta["TOT"]
    id16 = np.eye(128, dtype=np.float16)
    ones1 = np.ones((1, 128), np.float16)

    wext1 = make_wext(W1, a_src1, a_dst1).astype(np.float16)
    bext1 = make_bext(b1, a_src1, a_dst1).astype(np.float16).reshape(1, 264)
    nc0 = build_l0(meta)
    in0 = []
    for cd in meta["cores"]:
        xp = np.zeros((NW * 128, 128), np.float16)
        real = cd["perm"] >= 0
        xp[real] = x[cd["perm"][real]].astype(np.float16)
        in0.append({"xT": np.ascontiguousarray(xp.T), "Wext": wext1, "bext": bext1,
                    "ones1": ones1})
    r0 = _run(nc0, in0)
    houts = [r0.results[c]["hout"].reshape(NW * 128, 264) for c in range(NCORES)]

    nc_mid = build_agg(meta, last=False)
    nc_last = build_agg(meta, last=True)

    wexts = [make_wext(W2, a_src2, a_dst2).astype(np.float16),
             make_wext(W3, a_src3, a_dst3).astype(np.float16), None]
    bexts = [make_bext(b2, a_src2, a_dst2).astype(np.float16).reshape(1, 264),
             make_bext(b3, a_src3, a_dst3).astype(np.float16).reshape(1, 264), None]

    for li in range(3):
        last = li == 2
        # assemble global tables from per-core houts
        tab_h = np.zeros((TROWS, 256), np.float32)
        tab_as = np.zeros((TROWS, 4), np.float16)
        for cd, h in zip(meta["cores"], houts):
            real = cd["perm"] >= 0
            tab_h[cd["perm"][real]] = h[real, 0:256].astype(np.float32)
            tab_as[cd["perm"][real]] = h[real, 256:260]
        tab_as[ZROW] = np.float16(LOGIT_M[li])
        tab_as[PROW] = np.float16(-30000.0)
        tab8 = np.clip(tab_h, -240, 240).astype(ml_dtypes.float8_e4m3fn)

        ims = []
        for c, cd in enumerate(meta["cores"]):
            stream = tab8[cd["idx"]].reshape(128, TOT * 256)
            asst = tab_as[cd["idx"]].reshape(128, TOT * 4)
            adin = np.ascontiguousarray(
                houts[c].reshape(NW, 128, 264)[:, :, 260:264]
                .transpose(1, 0, 2).reshape(128, NW * 4)).astype(np.float16)
            im = {"stream": stream, "asst": asst, "adin": adin,
                  "mshift": np.full((128, 1), -LOGIT_M[li], np.float32),
                  "ident16": id16}
            if not last:
                W2c = wexts[li]  # [256, 264]
                im["Wext"] = np.ascontiguousarray(
                    np.concatenate([W2c[0:128, :], W2c[128:256, :]], axis=1))
                im["bext"] = bexts[li]
                im["ones1"] = ones1
            else:
                im["onehot"] = cd["pool_onehot"].astype(np.float16)
                im["fcw"] = np.tile(np.asarray(fc_W, np.float32).reshape(1, 256), (128, 1))
            ims.append(im)
        rr = _run(nc_mid if not last else nc_last, ims)
        if not last:
            houts = [rr.results[c]["hout"].reshape(NW * 128, 264) for c in range(NCORES)]
        else:
            outv = np.zeros(N_GRAPHS, np.float64)
            for c, cd in enumerate(meta["cores"]):
                pv = rr.results[c]["pout"].reshape(128)
                gb = cd["gbase"]
                hi = min(128, N_GRAPHS - gb)
                outv[gb:gb + hi] += pv[:hi]
            bias_fc = float(np.asarray(b3, np.float64) @ np.asarray(fc_W, np.float64).reshape(-1))
            outv += meta["gcounts"] * bias_fc
            outv += float(np.asarray(fc_b, np.float64).reshape(()))
    return outv.reshape(N_GRAPHS, 1).astype(np.float32)


# revision 5
# speedup vs baseline: 1.1295x; 1.0179x over previous
"""Trainium2 Bass kernel for 3-layer GAT + graph pooling (nn_GATModel).

Edge-stream design (v2): instead of on-device dma_gather (Q7 desc-gen wall at
~8.4ns/row), the HOST replicates table rows into a per-edge slot-aligned
stream between launches (pure index glue / byte permutation, no FP math on
activations). The device then:
  - loads the h-stream as fp8 via SWDGE cast-DMA (fp8 in HBM -> fp16 in SBUF)
  - computes per-edge softmax weights alpha = exp(lrelu(as+ad)-M)/den
  - expands alpha to full head width (rotating DVE-int32 / ACT / GpSimd)
  - msg = h * alpha_full as a flat fp16 DVE multiply (2x mode)
  - slot-sum via PE identity matmuls + GpSimd halving tree
  - epilogue: h_next = out @ [W|Msrc|Mdst] + bias-row (as/ad fused into the
    matmul via host-precomputed param-only matrices)
Four launches: L0 (x@W1ext), agg1, agg2, agg3+pool. Host between launches
does index-glue only (table assembly + per-edge replication + fp8 encode).
"""

import os
import numpy as np
import ml_dtypes

import concourse.bacc as bacc
import concourse.tile as tile
import concourse.mybir as mybir
from concourse import bass, bass_utils
from contextlib import ExitStack

F16 = mybir.dt.float16
F32 = mybir.dt.float32
F8 = mybir.dt.float8e4
I32 = mybir.dt.int32

N_NODES = 50000
N_EDGES = 800000
N_GRAPHS = 512
HEADS = 4
HDIM = 64
NCORES = 8
NEG_SLOPE = 0.2
LOGIT_M = [6.0, 10.0, 10.0]
ZROW = N_NODES          # table row: h=0, as=+M (safe self for empty slots)
PROW = N_NODES + 1      # table row: h=0, as=-30000 (e == 0 padding)
TROWS = N_NODES + 2

_EXEC_NS = []


def _trace_on():
    return bool(os.environ.get("GAT_TRACE"))


def _install_profhook():
    import sys, types
    if "antenv.axon_hooks" in sys.modules:
        return True
    try:
        mod = types.ModuleType("antenv.axon_hooks")
        state = {}
        mod.set_axon_ntff_profile_hook = lambda h: state.update(h=h)
        mod.get_axon_ntff_profile_hook = lambda: state.get("h")
        sys.modules["antenv.axon_hooks"] = mod
        sys.path.insert(0, "/root/.axon_site/trn_agent_boot")
        import trn_boot
        mod.set_axon_ntff_profile_hook(
            trn_boot._ntff_profile_via_ctypes("/opt/axon/libaxon_pjrt.so")
        )
        return True
    except Exception:
        sys.modules.pop("antenv.axon_hooks", None)
        return False


# ---------------------------------------------------------------- host prep

def build_meta(edge_index):
    src = np.asarray(edge_index[0], dtype=np.int64)
    dst = np.asarray(edge_index[1], dtype=np.int64)
    deg = np.bincount(dst, minlength=N_NODES)

    cum = np.cumsum(deg + 1)
    total = cum[-1]
    bounds = [0]
    for c in range(1, NCORES):
        bounds.append(int(np.searchsorted(cum, total * c / NCORES)))
    bounds.append(N_NODES)

    NW = max((bounds[c + 1] - bounds[c] + 127) // 128 for c in range(NCORES))
    maxn = NW * 128

    # edges sorted by dst; slot of edge within its dst segment
    order = np.argsort(dst, kind="stable")
    src_s, dst_s = src[order], dst[order]
    starts = np.searchsorted(dst_s, np.arange(N_NODES))
    slot_e = np.arange(len(src)) - starts[dst_s] + 1  # slot 0 = self

    cores = []
    kT_cores = []
    for c in range(NCORES):
        n0, n1 = bounds[c], bounds[c + 1]
        nodes = np.arange(n0, n1)
        o = np.argsort(deg[nodes], kind="stable")[::-1]
        perm = np.full(maxn, -1, np.int64)
        perm[: n1 - n0] = nodes[o]
        kc = np.zeros(NW, np.int32)
        dpad = np.zeros(maxn, np.int64)
        real = perm >= 0
        dpad[real] = deg[perm[real]] + 1
        kc = dpad.reshape(NW, 128).max(1).astype(np.int32)
        cores.append(dict(n0=n0, n1=n1, perm=perm))
        kT_cores.append(np.maximum(kc, 1))
    kT = np.maximum.reduce(kT_cores)
    offs = np.concatenate([[0], np.cumsum(kT)]).astype(np.int64)
    TOT = int(offs[-1])

    for cd in cores:
        perm = cd["perm"]
        posof = np.full(N_NODES, -1, np.int64)
        real = perm >= 0
        posof[perm[real]] = np.nonzero(real)[0]
        idx = np.full((128, TOT), PROW, np.int32)
        # self slots
        pos = posof[perm[real]]
        w, p = pos // 128, pos % 128
        idx[p, offs[w]] = perm[real]
        # empty partitions: self -> ZROW
        epos = np.nonzero(~real)[0]
        idx[epos % 128, offs[epos // 128]] = ZROW
        # edges owned by this core
        sel = (dst_s >= cd["n0"]) & (dst_s < cd["n1"])
        pos = posof[dst_s[sel]]
        w, p = pos // 128, pos % 128
        idx[p, offs[w] + slot_e[sel]] = src_s[sel].astype(np.int32)
        cd["idx"] = idx

    return dict(NW=NW, kT=kT, offs=offs, TOT=TOT, cores=cores)


def build_pool(meta, batch):
    batch = np.asarray(batch, dtype=np.int64)
    NW = meta["NW"]
    for cd in meta["cores"]:
        perm = cd["perm"]
        gbase = int(batch[cd["n0"]])
        gspan = int(batch[cd["n1"] - 1]) - gbase + 1
        assert gspan <= 128
        oh = np.zeros((NW * 128, 128), np.float16)
        real = perm >= 0
        oh[np.nonzero(real)[0], batch[perm[real]] - gbase] = 1.0
        cd["pool_onehot"] = oh.reshape(NW, 128, 128)
        cd["gbase"] = gbase
    meta["gcounts"] = np.bincount(batch, minlength=N_GRAPHS).astype(np.float64)


def make_wext(W, a_s, a_d):
    """[Fin, 256] params -> [Fin, 264] f32 with as/ad columns fused."""
    W = np.asarray(W, np.float64)
    a_s = np.asarray(a_s, np.float64)
    a_d = np.asarray(a_d, np.float64)
    W3 = W.reshape(W.shape[0], HEADS, HDIM)
    ms = np.einsum("chd,hd->ch", W3, a_s)
    md = np.einsum("chd,hd->ch", W3, a_d)
    return np.concatenate([W, ms, md], axis=1).astype(np.float32)


def make_bext(b, a_s, a_d):
    b = np.asarray(b, np.float64)
    bs = np.einsum("hd,hd->h", b.reshape(HEADS, HDIM), np.asarray(a_s, np.float64))
    bd = np.einsum("hd,hd->h", b.reshape(HEADS, HDIM), np.asarray(a_d, np.float64))
    return np.concatenate([b, bs, bd]).astype(np.float32)


# ---------------------------------------------------------------- device util

def _ap(t_ap, off_elems, dims):
    return bass.AP(t_ap.tensor, t_ap.offset + off_elems, dims)


# ---------------------------------------------------------------- programs

def build_l0(meta):
    NW = meta["NW"]
    nc = bacc.Bacc("TRN2", target_bir_lowering=False, debug=False, num_devices=NCORES)
    xT = nc.dram_tensor("xT", [128, NW * 128], F16, kind="ExternalInput").ap()
    Wext = nc.dram_tensor("Wext", [128, 264], F16, kind="ExternalInput").ap()
    bext = nc.dram_tensor("bext", [1, 264], F16, kind="ExternalInput").ap()
    ones1 = nc.dram_tensor("ones1", [1, 128], F16, kind="ExternalInput").ap()
    hout = nc.dram_tensor("hout", [NW, 128, 264], F16, kind="ExternalOutput").ap()

    with ExitStack() as ctx:
        tc = ctx.enter_context(tile.TileContext(nc))
        cpool = ctx.enter_context(tc.tile_pool(name="c", bufs=1))
        spool = ctx.enter_context(tc.tile_pool(name="s", bufs=4))
        pspool = ctx.enter_context(tc.tile_pool(name="ps", bufs=3, space="PSUM"))
        W_s = cpool.tile([128, 264], F16)
        nc.sync.dma_start(W_s[:], Wext[:])
        b_s = cpool.tile([1, 264], F16)
        nc.sync.dma_start(b_s[:], bext[:])
        o_s = cpool.tile([1, 128], F16)
        nc.sync.dma_start(o_s[:], ones1[:])
        xT_s = cpool.tile([128, NW * 128], F16)
        nc.sync.dma_start(xT_s[:], xT[:])
        for w in range(NW):
            hp = pspool.tile([128, 264], F32, tag="hp")
            nc.tensor.matmul(hp[:], lhsT=xT_s[:, w * 128:(w + 1) * 128], rhs=W_s[:],
                             start=True, stop=False)
            nc.tensor.matmul(hp[:], lhsT=o_s[:], rhs=b_s[:], start=False, stop=True)
            ho = spool.tile([128, 264], F16, tag="ho")
            nc.scalar.copy(ho[:], hp[:])
            nc.sync.dma_start(hout[w], ho[:])
    nc.compile()
    return nc


def build_agg(meta, last):
    NW, kT, offs, TOT = meta["NW"], meta["kT"], meta["offs"], meta["TOT"]
    nc = bacc.Bacc("TRN2", target_bir_lowering=False, debug=False, num_devices=NCORES)
    stream = nc.dram_tensor("stream", [128, TOT * 256], F8, kind="ExternalInput").ap()
    asst = nc.dram_tensor("asst", [128, TOT * 4], F16, kind="ExternalInput").ap()
    adin = nc.dram_tensor("adin", [128, NW * 4], F16, kind="ExternalInput").ap()
    mshift = nc.dram_tensor("mshift", [128, 1], F32, kind="ExternalInput").ap()
    ident16 = nc.dram_tensor("ident16", [128, 128], F16, kind="ExternalInput").ap()
    if not last:
        Wext = nc.dram_tensor("Wext", [128, 2 * 264], F16, kind="ExternalInput").ap()
        bext = nc.dram_tensor("bext", [1, 264], F16, kind="ExternalInput").ap()
        ones1 = nc.dram_tensor("ones1", [1, 128], F16, kind="ExternalInput").ap()
        hout = nc.dram_tensor("hout", [NW, 128, 264], F16, kind="ExternalOutput").ap()
    else:
        onehot = nc.dram_tensor("onehot", [NW, 128, 128], F16, kind="ExternalInput").ap()
        fcw = nc.dram_tensor("fcw", [128, 256], F32, kind="ExternalInput").ap()
        pout = nc.dram_tensor("pout", [128, 1], F32, kind="ExternalOutput").ap()

    kmax = int(kT.max())

    with ExitStack() as ctx:
        tc = ctx.enter_context(tile.TileContext(nc))
        cpool = ctx.enter_context(tc.tile_pool(name="c", bufs=1))
        gpool = ctx.enter_context(tc.tile_pool(name="g", bufs=5))
        spool = ctx.enter_context(tc.tile_pool(name="s", bufs=6))
        pspool = ctx.enter_context(tc.tile_pool(name="ps", bufs=2, space="PSUM"))
        pxpool = ctx.enter_context(tc.tile_pool(name="px", bufs=3, space="PSUM"))
        pppool = ctx.enter_context(tc.tile_pool(name="pp", bufs=1, space="PSUM"))

        as_s = cpool.tile([128, TOT * 4], F16)
        nc.sync.dma_start(as_s[:], asst[:])
        ad_s = cpool.tile([128, NW * 4], F16)
        nc.sync.dma_start(ad_s[:], adin[:])
        msh_s = cpool.tile([128, 1], F32)
        nc.sync.dma_start(msh_s[:], mshift[:])
        id16 = cpool.tile([128, 128], F16)
        nc.sync.dma_start(id16[:], ident16[:])
        alslope = cpool.tile([128, 1], F32)
        nc.vector.memset(alslope[:], NEG_SLOPE)
        if not last:
            W_s = cpool.tile([128, 2 * 264], F16)
            nc.sync.dma_start(W_s[:], Wext[:])
            b_s = cpool.tile([1, 264], F16)
            nc.sync.dma_start(b_s[:], bext[:])
            o_s = cpool.tile([1, 128], F16)
            nc.sync.dma_start(o_s[:], ones1[:])
        else:
            fcw_s = cpool.tile([128, 256], F32)
            nc.sync.dma_start(fcw_s[:], fcw[:])
            pp = pppool.tile([128, 256], F32)

        asc = sorted(range(NW), key=lambda i: int(kT[i]))
        worder = asc[0::2] + asc[1::2][::-1]
        for wi, w in enumerate(worder):
            k = int(kT[w])
            o4 = int(offs[w]) * 4
            # h-stream: fp8 in HBM -> fp16 in SBUF via SWDGE cast DMA
            g = gpool.tile([128, k * 256], F16, tag="g")
            nc.gpsimd.dma_start(g[:], stream[:, int(offs[w]) * 256:(int(offs[w]) + k) * 256])

            # logits lg = as + ad (f32), lr = leaky_relu
            lg = spool.tile([128, kmax * 4], F32, tag="lg")
            as_ap = as_s[:, o4:o4 + k * 4]
            ad_b = _ap(ad_s[:], w * 4, [list(ad_s[:].ap[0]), [0, k], [1, 4]])
            as_v = _ap(as_s[:], o4, [list(as_s[:].ap[0]), [4, k], [1, 4]])
            nc.vector.tensor_tensor(
                out=_ap(lg[:], 0, [list(lg[:].ap[0]), [4, k], [1, 4]]),
                in0=as_v, in1=ad_b, op=mybir.AluOpType.add)
            lr = spool.tile([128, kmax * 4], F32, tag="lr")
            nc.scalar.activation(lr[:, :k * 4], lg[:, :k * 4],
                                 mybir.ActivationFunctionType.Prelu,
                                 alpha=alslope[:])
            # e = exp(lr - M)  (ACT)
            e = spool.tile([128, kmax * 4], F16, tag="e")
            nc.scalar.activation(e[:, :k * 4], lr[:, :k * 4],
                                 mybir.ActivationFunctionType.Exp,
                                 bias=msh_s[:], scale=1.0)
            # den + recip
            den = spool.tile([128, 4], F32, tag="den")
            nc.vector.reduce_sum(
                den[:], _ap(e[:], 0, [list(e[:].ap[0]), [1, 4], [4, k]]),
                axis=mybir.AxisListType.X)
            rdeb = spool.tile([128, 4], F32, tag="rdeb")
            nc.vector.reciprocal(rdeb[:], den[:])
            # etwin: alpha duplicated pairwise = e * recip (one 3-dim TT)
            etw = spool.tile([128, kmax * 8], F16, tag="etw")
            rdeb_b3 = _ap(rdeb[:], 0, [list(rdeb[:].ap[0]), [0, k], [1, 4], [0, 2]])
            e_v3 = _ap(e[:], 0, [list(e[:].ap[0]), [4, k], [1, 4], [0, 2]])
            nc.vector.tensor_tensor(
                out=_ap(etw[:], 0, [list(etw[:].ap[0]), [8, k], [2, 4], [1, 2]]),
                in0=e_v3, in1=rdeb_b3, op=mybir.AluOpType.mult)
            # msg = h * alpha, in place: pair-AP trick keeps DVE 2x mode
            # (innermost step-1 over the duplicated alpha pairs in etw)
            ge = _ap(g[:], 0, [list(g[:].ap[0]), [64, k * 4], [2, 32], [1, 2]])
            ee = _ap(etw[:], 0, [list(etw[:].ap[0]), [2, k * 4], [0, 32], [1, 2]])
            nc.vector.tensor_tensor(out=ge, in0=ge, in1=ee, op=mybir.AluOpType.mult)
            # slot-sum: GpSimd tree chunk + PE matmuls
            kG = max(k // 5, 2) if k >= 10 else 0
            g3 = g[:].rearrange("p (k d) -> p k d", d=256)
            if kG:
                a0, n = k - kG, kG
                while n > 1:
                    h = n // 2
                    nc.gpsimd.tensor_tensor(
                        out=g[:, a0 * 256:(a0 + h) * 256],
                        in0=g[:, a0 * 256:(a0 + h) * 256],
                        in1=g[:, (a0 + n - h) * 256:(a0 + n) * 256],
                        op=mybir.AluOpType.add)
                    n -= h
            kPE = k - kG + (1 if kG else 0)
            ps = pspool.tile([128, 256], F32, tag="ps")
            for t in range(kPE):
                nc.tensor.matmul(ps[:], lhsT=id16[:], rhs=g3[:, t if t < k - kG else k - kG, :],
                                 start=(t == 0), stop=(t == kPE - 1))
            # out1 = ps (already alpha-normalized); cast f16
            of = spool.tile([128, 256], F16, tag="of")
            nc.scalar.copy(of[:], ps[:])
            if not last:
                outT = spool.tile([128, 256], F16, tag="outT")
                for q in range(2):
                    pt = pxpool.tile([128, 128], F16, tag="pt")
                    nc.tensor.transpose(pt[:], of[:, q * 128:(q + 1) * 128], id16[:])
                    nc.scalar.copy(outT[:, q * 128:(q + 1) * 128], pt[:])
                hp = pxpool.tile([128, 264], F32, tag="hp")
                for q in range(2):
                    nc.tensor.matmul(hp[:], lhsT=outT[:, q * 128:(q + 1) * 128],
                                     rhs=W_s[:, q * 264:(q + 1) * 264],
                                     start=(q == 0), stop=False)
                nc.tensor.matmul(hp[:], lhsT=o_s[:], rhs=b_s[:], start=False, stop=True)
                ho = spool.tile([128, 264], F16, tag="ho")
                nc.scalar.copy(ho[:], hp[:])
                nc.sync.dma_start(hout[w], ho[:])
            else:
                ohw = spool.tile([128, 128], F16, tag="ohw")
                nc.sync.dma_start(ohw[:], onehot[w])
                nc.tensor.matmul(pp[:], lhsT=ohw[:], rhs=of[:],
                                 start=(wi == 0), stop=(wi == NW - 1),
                                 skip_group_check=True)
        if last:
            fm = spool.tile([128, 256], F32, tag="fm")
            nc.vector.tensor_tensor(out=fm[:], in0=pp[:], in1=fcw_s[:],
                                    op=mybir.AluOpType.mult)
            pv = spool.tile([128, 1], F32, tag="pv")
            nc.vector.reduce_sum(pv[:], fm[:], axis=mybir.AxisListType.X)
            nc.sync.dma_start(pout[:], pv[:])
    nc.compile()
    return nc


# ---------------------------------------------------------------- run

def _run(nc, in_maps):
    trace = _trace_on() and _install_profhook()
    res = bass_utils.run_bass_kernel_spmd(
        nc, in_maps=in_maps, core_ids=list(range(NCORES)), trace=trace
    )
    if _trace_on():
        _EXEC_NS.append(res.exec_time_ns)
    return res


def kernel(x, edge_index, batch, W1, a_src1, a_dst1, b1, W2, a_src2, a_dst2, b2,
           W3, a_src3, a_dst3, b3, fc_W, fc_b):
    _EXEC_NS.clear()
    x = np.asarray(x, np.float32)
    edge_index = np.asarray(edge_index)
    batch = np.asarray(batch)
    meta = build_meta(edge_index)
    build_pool(meta, batch)
    NW, TOT = meta["NW"], meta["TOT"]
    id16 = np.eye(128, dtype=np.float16)
    ones1 = np.ones((1, 128), np.float16)

    wext1 = make_wext(W1, a_src1, a_dst1).astype(np.float16)
    bext1 = make_bext(b1, a_src1, a_dst1).astype(np.float16).reshape(1, 264)
    nc0 = build_l0(meta)
    in0 = []
    for cd in meta["cores"]:
        xp = np.zeros((NW * 128, 128), np.float16)
        real = cd["perm"] >= 0
        xp[real] = x[cd["perm"][real]].astype(np.float16)
        in0.append({"xT": np.ascontiguousarray(xp.T), "Wext": wext1, "bext": bext1,
                    "ones1": ones1})
    r0 = _run(nc0, in0)
    houts = [r0.results[c]["hout"].reshape(NW * 128, 264) for c in range(NCORES)]

    nc_mid = build_agg(meta, last=False)
    nc_last = build_agg(meta, last=True)

    wexts = [make_wext(W2, a_src2, a_dst2).astype(np.float16),
             make_wext(W3, a_src3, a_dst3).astype(np.float16), None]
    bexts = [make_bext(b2, a_src2, a_dst2).astype(np.float16).reshape(1, 264),
             make_bext(b3, a_src3, a_dst3).astype(np.float16).reshape(1, 264), None]

    for li in range(3):
        last = li == 2
        # assemble global tables from per-core houts
        tab_h = np.zeros((TROWS, 256), np.float32)
        tab_as = np.zeros((TROWS, 4), np.float16)
        for cd, h in zip(meta["cores"], houts):
            real = cd["perm"] >= 0
            tab_h[cd["perm"][real]] = h[real, 0:256].astype(np.float32)
            tab_as[cd["perm"][real]] = h[real, 256:260]
        tab_as[ZROW] = np.float16(LOGIT_M[li])
        tab_as[PROW] = np.float16(-30000.0)
        tab8 = np.clip(tab_h, -240, 240).astype(ml_dtypes.float8_e4m3fn)

        ims = []
        for c, cd in enumerate(meta["cores"]):
            stream = tab8[cd["idx"]].reshape(128, TOT * 256)
            asst = tab_as[cd["idx"]].reshape(128, TOT * 4)
            adin = np.ascontiguousarray(
                houts[c].reshape(NW, 128, 264)[:, :, 260:264]
                .transpose(1, 0, 2).reshape(128, NW * 4)).astype(np.float16)
            im = {"stream": stream, "asst": asst, "adin": adin,
                  "mshift": np.full((128, 1), -LOGIT_M[li], np.float32),
                  "ident16": id16}
            if not last:
                W2c = wexts[li]  # [256, 264]
                im["Wext"] = np.ascontiguousarray(
                    np.concatenate([W2c[0:128, :], W2c[128:256, :]], axis=1))
                im["bext"] = bexts[li]
                im["ones1"] = ones1
            else:
                im["onehot"] = cd["pool_onehot"].astype(np.float16)
                im["fcw"] = np.tile(np.asarray(fc_W, np.float32).reshape(1, 256), (128, 1))
            ims.append(im)
        rr = _run(nc_mid if not last else nc_last, ims)
        if not last:
            houts = [rr.results[c]["hout"].reshape(NW * 128, 264) for c in range(NCORES)]
        else:
            outv = np.zeros(N_GRAPHS, np.float64)
            for c, cd in enumerate(meta["cores"]):
                pv = rr.results[c]["pout"].reshape(128)
                gb = cd["gbase"]
                hi = min(128, N_GRAPHS - gb)
                outv[gb:gb + hi] += pv[:hi]
            bias_fc = float(np.asarray(b3, np.float64) @ np.asarray(fc_W, np.float64).reshape(-1))
            outv += meta["gcounts"] * bias_fc
            outv += float(np.asarray(fc_b, np.float64).reshape(()))
    return outv.reshape(N_GRAPHS, 1).astype(np.float32)
